# revision 5
# baseline (speedup 1.0000x reference)
"""Causal self-attention (single head) on 8 TRN2 NeuronCores.

Sharding: data-parallel over batch (4) x query-interleave (2).
Core c handles batch b = c//2 and the 8 query blocks (128 q each)
J_BLOCKS[c%2]; slot t's block g is in {2t, 2t+1}, so covering key
tiles 0..2t+1 (natural global order) is uniform across cores and the
causal mask beyond the static structure is data-driven.

Algorithm per core (all matmuls bf16 with f32 PSUM):
  QT[d2, q]  = Wqk^T-projected queries (Wqk = Wq^T Wk host-fused), so
               scores = QT^T x_k^T needs no on-chip K projection.
  V'[k, eo]  = x Wvo^T (Wvo = Wo Wv host-fused): attended @ V' IS the
               final output projection.
  scoresT[k, q] computed per key tile kt with q spanning all slots
               that cover kt -> wide matmuls, and softmax weights come
               out already in [k, q] layout: no transposes at all.
  softmax    = exp without max subtraction (scores ~ N(0,1), safe in
               f32), denominator via ones-column matmul, normalization
               folded into the attended PSUM eviction scale.

USE_CC: V' is computed only for this core's own 8 key blocks and
pair-wise AllGathered (cores 2b/2b+1), halving the biggest matmul.
"""

from contextlib import ExitStack

import numpy as np
import ml_dtypes

USE_CC = True

B, S, D = 4, 2048, 1024
P = 128
ND = D // P  # 8 contraction chunks
NE = D // P  # 8 feature chunks
NSK = S // P  # 16 key tiles
NQB = 8  # query blocks per core
SQH = NQB * P  # 1024 queries per core
J_BLOCKS = (
    [0, 3, 4, 7, 8, 11, 12, 15],
    [1, 2, 5, 6, 9, 10, 13, 14],
)
VG_BLOCK = J_BLOCKS[0] + J_BLOCKS[1]  # allgather row-block i holds block VG_BLOCK[i]
VG_POS = {g: i for i, g in enumerate(VG_BLOCK)}
SCALE = 1.0 / np.sqrt(np.float32(D))  # 1/32
NEG_BIG = -1.0e30
CPW = SQH + NSK  # packed f32 consts width

_NC = None


def _emit(nc, tc, dr, out_d):
    from concourse import mybir

    BF = mybir.dt.bfloat16
    F32 = mybir.dt.float32
    AF = mybir.ActivationFunctionType
    Alu = mybir.AluOpType

    with ExitStack() as ctx:
        const = ctx.enter_context(tc.tile_pool(name="const", bufs=1))
        cpak = const.tile([P, CPW], F32)
        nc.gpsimd.dma_start(cpak[:], dr["cpak"])
        qpos = cpak[:, 0:SQH]
        kposc = cpak[:, SQH : SQH + NSK]
        cbf = const.tile([P, 8], BF)
        nc.gpsimd.dma_start(cbf[:], dr["cbf"])
        ones1 = cbf[:, 0:1]

        # persistent activation storage
        xq_pool = ctx.enter_context(tc.tile_pool(name="xq", bufs=ND))
        xt_pool = ctx.enter_context(tc.tile_pool(name="xt", bufs=ND))
        qt_pool = ctx.enter_context(tc.tile_pool(name="qt", bufs=NE))
        v_pool = ctx.enter_context(tc.tile_pool(name="v", bufs=NSK))
        XQ, XT, QT, V = [None] * ND, [None] * ND, [], [None] * NSK

        # ---------------- phase A ----------------
        with ExitStack() as actx:
            wv_pool = actx.enter_context(tc.tile_pool(name="wv", bufs=ND))
            wq_pool = actx.enter_context(tc.tile_pool(name="wq", bufs=ND))
            WV, WQ = [None] * ND, [None] * ND

            # input streams on the 3 DMA rings (sync/scalar/gpsimd):
            # xq + wv first, interleaved by need order (V d-step d needs
            # both), then wq (QT at ~30us), then xT (scores at ~57us)
            RINGS = (nc.sync, nc.scalar, nc.gpsimd)

            def load(lst, d, pool, w, key, ring):
                t = pool.tile([P, w], BF, name=key)
                RINGS[ring].dma_start(t[:], dr[key][d])
                lst[d] = t

            for d in range(ND):
                load(XQ, d, xq_pool, SQH, "xq", d % 3)
                load(WV, d, wv_pool, D, "wvT", (d + 1) % 3)
            for d in range(ND):
                load(WQ, d, wq_pool, D, "wq2", d % 2)
            for d in range(ND):
                load(XT, d, xt_pool, S, "xT", 2)

            if USE_CC:
                # ---- V' for my 8 blocks (from xq), d-outer in 2 passes ----
                dram = actx.enter_context(
                    tc.tile_pool(name="dram", bufs=1, space="DRAM")
                )
                vin = dram.tile([NQB * P, D], BF)
                vg = dram.tile([2 * NQB * P, D], BF)
                aps = actx.enter_context(
                    tc.tile_pool(name="aps", bufs=8, space="PSUM")
                )
                vsb = actx.enter_context(tc.tile_pool(name="vsb", bufs=NQB))
                for half in range(2):
                    pss = [
                        [aps.tile([P, 512], F32, tag="ps", name="ps") for _ in range(2)]
                        for _ in range(4)
                    ]
                    for d in range(ND):
                        for si in range(4):
                            s = half * 4 + si
                            for nt in range(2):
                                nc.tensor.matmul(
                                    pss[si][nt][:],
                                    XQ[d][:, s * P : (s + 1) * P],
                                    WV[d][:, nt * 512 : (nt + 1) * 512],
                                    start=(d == 0),
                                    stop=(d == ND - 1),
                                )
                    for si in range(4):
                        s = half * 4 + si
                        vt = vsb.tile([P, D], BF, name="vsb")
                        for nt in range(2):
                            nc.scalar.activation(
                                vt[:, nt * 512 : (nt + 1) * 512],
                                pss[si][nt][:],
                                AF.Copy,
                            )
                        nc.sync.dma_start(vin[s * P : (s + 1) * P, :], vt[:])
                nc.gpsimd.collective_compute(
                    "AllGather",
                    mybir.AluOpType.bypass,
                    replica_groups=[[0, 1], [2, 3], [4, 5], [6, 7]],
                    ins=[vin.opt()],
                    outs=[vg.opt()],
                )
                # readback in ascending global-block order (attend order)
                for g in range(NSK):
                    i = VG_POS[g]
                    vt = v_pool.tile([P, D], BF, name="v")
                    (nc.gpsimd if g % 2 == 0 else nc.scalar).dma_start(
                        vt[:], vg[i * P : (i + 1) * P, :]
                    )
                    V[g] = vt

                # ---- QT, e-outer (xq fully resident by now) ----
                for e in range(NE):
                    qts = qt_pool.tile([P, SQH], BF, name="qt")
                    for h in range(2):
                        qp = aps.tile([P, 512], F32, tag="ps", name="ps")
                        for d in range(ND):
                            nc.tensor.matmul(
                                qp[:],
                                WQ[d][:, e * P : (e + 1) * P],
                                XQ[d][:, h * 512 : (h + 1) * 512],
                                start=(d == 0),
                                stop=(d == ND - 1),
                            )
                        nc.scalar.activation(
                            qts[:, h * 512 : (h + 1) * 512], qp[:], AF.Copy
                        )
                    QT.append(qts)
            else:
                # ---- QT first (d-outer, 2 q-passes, streams xq+wq) ----
                aps = actx.enter_context(
                    tc.tile_pool(name="aps", bufs=8, space="PSUM")
                )
                for h in range(2):
                    pss = [aps.tile([P, 512], F32, tag="ps", name="ps") for _ in range(NE)]
                    for d in range(ND):
                        for e in range(NE):
                            nc.tensor.matmul(
                                pss[e][:],
                                WQ[d][:, e * P : (e + 1) * P],
                                XQ[d][:, h * 512 : (h + 1) * 512],
                                start=(d == 0),
                                stop=(d == ND - 1),
                            )
                    for e in range(NE):
                        if h == 0:
                            QT.append(qt_pool.tile([P, SQH], BF, name="qt"))
                        nc.scalar.activation(
                            QT[e][:, h * 512 : (h + 1) * 512], pss[e][:], AF.Copy
                        )
                # ---- V' for all 16 tiles from xT, s-outer ----
                for s in range(NSK):
                    vt = v_pool.tile([P, D], BF, name="v")
                    pss = [aps.tile([P, 512], F32, tag="ps", name="ps") for _ in range(2)]
                    for d in range(ND):
                        for nt in range(2):
                            nc.tensor.matmul(
                                pss[nt][:],
                                XT[d][:, s * P : (s + 1) * P],
                                WV[d][:, nt * 512 : (nt + 1) * 512],
                                start=(d == 0),
                                stop=(d == ND - 1),
                            )
                    for nt in range(2):
                        nc.scalar.activation(
                            vt[:, nt * 512 : (nt + 1) * 512], pss[nt][:], AF.Copy
                        )
                    V[s] = vt

        # ---------------- phase B: scores + attend ----------------
        with ExitStack() as bctx:
            maskp = bctx.enter_context(tc.tile_pool(name="mask", bufs=2))
            smp = bctx.enter_context(tc.tile_pool(name="sm", bufs=2))
            expA = bctx.enter_context(tc.tile_pool(name="expA", bufs=8))
            expB = bctx.enter_context(tc.tile_pool(name="expB", bufs=8))
            statp = bctx.enter_context(tc.tile_pool(name="stat", bufs=2))
            outp = bctx.enter_context(tc.tile_pool(name="out", bufs=2))
            ps_s = bctx.enter_context(tc.tile_pool(name="ps_s", bufs=4, space="PSUM"))
            ps_a = bctx.enter_context(tc.tile_pool(name="ps_a", bufs=2, space="PSUM"))
            denp = bctx.enter_context(tc.tile_pool(name="den", bufs=2, space="PSUM"))
            EXP = [None] * NSK

            def emit_scores(g):
                t0 = g // 2
                span = (NQB - t0) * P
                qoff = t0 * P
                # additive causal mask for the slot-t0 q-slice of this tile
                mk = maskp.tile([P, P], F32, tag="mk", name="mk")
                nc.vector.tensor_scalar(
                    mk[:],
                    qpos[:, qoff : qoff + P],
                    kposc[:, g : g + 1],
                    NEG_BIG,
                    op0=Alu.is_lt,
                    op1=Alu.mult,
                )
                pool = expA if span > 512 else expB
                ex = pool.tile([P, max(span, 512)], BF, name="ex")
                EXP[g] = ex
                for off in range(0, span, 512):
                    w = min(512, span - off)
                    ps = ps_s.tile([P, 512], F32, tag="ps", name="ps")
                    for e in range(NE):
                        nc.tensor.matmul(
                            ps[:, 0:w],
                            XT[e][:, g * P : (g + 1) * P],
                            QT[e][:, qoff + off : qoff + off + w],
                            start=(e == 0),
                            stop=(e == NE - 1),
                        )
                    if off == 0:
                        sm = smp.tile([P, P], F32, tag="sm", name="sm")
                        nc.vector.tensor_tensor(sm[:], ps[:, 0:P], mk[:], op=Alu.add)
                        nc.scalar.activation(
                            ex[:, 0:P], sm[:], AF.Exp, scale=float(SCALE)
                        )
                        if w > P:
                            nc.scalar.activation(
                                ex[:, P:w], ps[:, P:w], AF.Exp, scale=float(SCALE)
                            )
                    else:
                        nc.scalar.activation(
                            ex[:, off : off + w],
                            ps[:, 0:w],
                            AF.Exp,
                            scale=float(SCALE),
                        )

            def emit_attend(t):
                nk = 2 * t + 2
                dps = denp.tile([P, 1], F32, tag="dp", name="dp")
                pas = [ps_a.tile([P, 512], F32, tag="pa", name="pa") for _ in range(2)]
                for g2 in range(nk):
                    lt = EXP[g2][:, (t - g2 // 2) * P : (t - g2 // 2 + 1) * P]
                    nc.tensor.matmul(
                        dps[:], lt, ones1, start=(g2 == 0), stop=(g2 == nk - 1)
                    )
                    for nt in range(2):
                        nc.tensor.matmul(
                            pas[nt][:],
                            lt,
                            V[g2][:, nt * 512 : (nt + 1) * 512],
                            start=(g2 == 0),
                            stop=(g2 == nk - 1),
                        )
                rinv = statp.tile([P, 1], F32, tag="ri", name="ri")
                nc.vector.reciprocal(rinv[:], dps[:])
                ob = outp.tile([P, D], F32, tag="ob", name="ob")
                for nt in range(2):
                    nc.scalar.activation(
                        ob[:, nt * 512 : (nt + 1) * 512],
                        pas[nt][:],
                        AF.Copy,
                        scale=rinv[:],
                    )
                    nc.sync.dma_start(
                        out_d[t][:, nt * 512 : (nt + 1) * 512],
                        ob[:, nt * 512 : (nt + 1) * 512],
                    )

            # one-tile lookahead: A(t) emitted after S(2t+2) so the exp
            # eviction of S(2t+1) hides under S(2t+2)'s matmuls
            for g in range(NSK):
                emit_scores(g)
                if g >= 2 and g % 2 == 0:
                    emit_attend(g // 2 - 1)
            emit_attend(NQB - 1)


def build_nc():
    """Build + compile the SPMD Bass program (cached)."""
    global _NC
    if _NC is not None:
        return _NC
    from concourse import bacc, mybir
    import concourse.tile as tile

    BF = mybir.dt.bfloat16
    F32 = mybir.dt.float32

    nc = bacc.Bacc(
        "TRN2",
        target_bir_lowering=False,
        debug=False,
        enable_asserts=False,
        num_devices=8,
    )
    dr = {}

    def din(name, shape, dt):
        dr[name] = nc.dram_tensor(name, shape, dt, kind="ExternalInput").ap()

    din("xT", (ND, P, S), BF)
    din("xq", (ND, P, SQH), BF)
    din("wq2", (ND, P, D), BF)
    din("wvT", (ND, P, D), BF)
    din("cpak", (P, CPW), F32)
    din("cbf", (P, 8), BF)
    out_d = nc.dram_tensor("out_c", (NQB, P, D), F32, kind="ExternalOutput").ap()

    with tile.TileContext(nc) as tc:
        _emit(nc, tc, dr, out_d)
    nc.compile()
    _NC = nc
    return nc


def make_in_maps(x, Wq, bq, Wk, bk, Wv, bv, Wo, bo):
    """Host-side sharding: per-core input dicts (bf16 compute operands)."""
    bf16 = ml_dtypes.bfloat16
    f32 = np.float32

    # host-fused weights (f32 GEMMs, exact up to fp32):
    #   scores = (x Wq^T)(x Wk^T)^T = x (Wq^T Wk) x^T       -> Wqk
    #   out    = softmax(..) (x Wv^T) Wo^T = softmax(..) x (Wo Wv)^T
    # Requires bq = bk = 0 (guaranteed by the problem spec).
    Wqk = Wq.T.astype(np.float32) @ Wk.astype(np.float32)  # [d1, d2]
    Wvo = Wo.astype(np.float32) @ Wv.astype(np.float32)  # [eo, d]
    wq2 = np.ascontiguousarray(Wqk.reshape(ND, P, D)).astype(bf16)
    wvT = np.ascontiguousarray(Wvo.T.reshape(ND, P, D)).astype(bf16)
    kposc = (np.arange(NSK, dtype=f32) * P)[None, :] + np.arange(P, dtype=f32)[
        :, None
    ]  # [P, NSK]: kposc[p, g] = g*128 + p
    cbf = np.ones((P, 8), dtype=bf16)

    in_maps = []
    for c in range(8):
        b, j = c // 2, c % 2
        blocks = J_BLOCKS[j]
        xTb = np.ascontiguousarray(x[b].T)  # [D, S] natural key order
        qcols = np.concatenate(
            [np.r_[P * g : P * (g + 1)] for g in blocks]
        )
        xqb = np.ascontiguousarray(xTb[:, qcols])  # [D, SQH]
        qpos = np.broadcast_to(qcols.astype(f32), (P, SQH))
        cpak = np.concatenate([qpos, kposc], axis=1)
        in_maps.append(
            {
                "xT": xTb.reshape(ND, P, S).astype(bf16),
                "xq": xqb.reshape(ND, P, SQH).astype(bf16),
                "wq2": wq2,
                "wvT": wvT,
                "cpak": np.ascontiguousarray(cpak.astype(f32)),
                "cbf": cbf,
            }
        )
    return in_maps


def assemble_out(results, bvo):
    out = np.empty((B, S, D), dtype=np.float32)
    for c in range(8):
        b, j = c // 2, c % 2
        blocks = J_BLOCKS[j]
        oc = results[c]["out_c"]  # (8, 128, 1024)
        for t, g in enumerate(blocks):
            out[b, P * g : P * (g + 1), :] = oc[t]
    if bvo is not None:
        out += bvo[None, None, :]
    return out


def kernel(x, Wq, bq, Wk, bk, Wv, bv, Wo, bo):
    from concourse.bass_utils import run_bass_kernel_spmd

    nc = build_nc()
    in_maps = make_in_maps(x, Wq, bq, Wk, bk, Wv, bv, Wo, bo)
    res = run_bass_kernel_spmd(nc, in_maps, core_ids=list(range(8)))
    bvo = Wo.astype(np.float32) @ bv.astype(np.float32) + bo.astype(np.float32)
    return assemble_out(res.results, bvo)


# revision 8
# speedup vs baseline: 1.1541x; 1.1541x over previous
"""Causal self-attention (single head) on 8 TRN2 NeuronCores.

Sharding: data-parallel over batch (4) x query-interleave (2).
Core c handles batch b = c//2 and the 8 query blocks (128 q each)
J_BLOCKS[c%2]; slot t's block g is in {2t, 2t+1}, so covering key
tiles 0..2t+1 (natural global order) is uniform across cores and the
causal mask beyond the static structure is data-driven.

Algorithm per core (all matmuls bf16 with f32 PSUM):
  QT[d2, q]  = Wqk^T-projected queries (Wqk = Wq^T Wk host-fused), so
               scores = QT^T x_k^T needs no on-chip K projection.
  V'[k, eo]  = x Wvo^T (Wvo = Wo Wv host-fused): attended @ V' IS the
               final output projection.
  scoresT[k, q] computed per key tile kt with q spanning all slots
               that cover kt -> wide matmuls, and softmax weights come
               out already in [k, q] layout: no transposes at all.
  softmax    = exp without max subtraction (scores ~ N(0,1), safe in
               f32), denominator via ones-column matmul, normalization
               folded into the attended PSUM eviction scale.

USE_CC: V' is computed only for this core's own 8 key blocks and
pair-wise AllGathered (cores 2b/2b+1), halving the biggest matmul.
"""

from contextlib import ExitStack

import numpy as np
import ml_dtypes

USE_CC = True

B, S, D = 4, 2048, 1024
P = 128
ND = D // P  # 8 contraction chunks
NE = D // P  # 8 feature chunks
NSK = S // P  # 16 key tiles
NQB = 8  # query blocks per core
SQH = NQB * P  # 1024 queries per core
J_BLOCKS = (
    [0, 3, 4, 7, 8, 11, 12, 15],
    [1, 2, 5, 6, 9, 10, 13, 14],
)
# per-half allgather row order: [my 4 blocks (rank 0) | peer 4 (rank 1)]
VG_BLOCK = (
    J_BLOCKS[0][:4] + J_BLOCKS[1][:4] + J_BLOCKS[0][4:] + J_BLOCKS[1][4:]
)
VG_POS = {g: i for i, g in enumerate(VG_BLOCK)}
SCALE = 1.0 / np.sqrt(np.float32(D))  # 1/32
NEG_BIG = -1.0e30
CPW = SQH + NSK  # packed f32 consts width

_NC = None


def _emit(nc, tc, dr, out_d):
    from concourse import mybir

    BF = mybir.dt.bfloat16
    F32 = mybir.dt.float32
    AF = mybir.ActivationFunctionType
    Alu = mybir.AluOpType

    with ExitStack() as ctx:
        const = ctx.enter_context(tc.tile_pool(name="const", bufs=1))
        cpak = const.tile([P, CPW], F32)
        nc.gpsimd.dma_start(cpak[:], dr["cpak"])
        qpos = cpak[:, 0:SQH]
        kposc = cpak[:, SQH : SQH + NSK]
        cbf = const.tile([P, 8], BF)
        nc.gpsimd.dma_start(cbf[:], dr["cbf"])
        ones1 = cbf[:, 0:1]

        # persistent activation storage
        xq_pool = ctx.enter_context(tc.tile_pool(name="xq", bufs=ND))
        xt_pool = ctx.enter_context(tc.tile_pool(name="xt", bufs=ND))
        qt_pool = ctx.enter_context(tc.tile_pool(name="qt", bufs=NE))
        v_pool = ctx.enter_context(tc.tile_pool(name="v", bufs=NSK))
        XQ, XT, QT, V = [None] * ND, [None] * ND, [], [None] * NSK

        # ---------------- phase A ----------------
        with ExitStack() as actx:
            wv_pool = actx.enter_context(tc.tile_pool(name="wv", bufs=ND))
            wq_pool = actx.enter_context(tc.tile_pool(name="wq", bufs=ND))
            WV, WQ = [None] * ND, [None] * ND

            # input streams on the 3 DMA rings (sync/scalar/gpsimd):
            # xq + wv first, interleaved by need order (V d-step d needs
            # both), then wq (QT at ~30us), then xT (scores at ~57us)
            RINGS = (nc.sync, nc.scalar, nc.gpsimd)

            def load(lst, d, pool, w, key, ring):
                t = pool.tile([P, w], BF, name=key)
                RINGS[ring].dma_start(t[:], dr[key][d])
                lst[d] = t

            for d in range(ND):
                load(XQ, d, xq_pool, SQH, "xq", d % 3)
                load(WV, d, wv_pool, D, "wvT", (d + 1) % 3)
            for d in range(ND):
                load(WQ, d, wq_pool, D, "wq2", d % 2)
            for d in range(ND):
                load(XT, d, xt_pool, S, "xT", 1)

            if USE_CC:
                # ---- V' for my 8 blocks (from xq), d-outer in 2 passes.
                # Each pass's 4 blocks, paired with the peer's, are exactly
                # global key tiles 0..7 (pass 0) / 8..15 (pass 1), so each
                # pass feeds its own AllGather: the first gather is in
                # flight while pass 1 + QT still compute. ----
                dram = actx.enter_context(
                    tc.tile_pool(name="dram", bufs=1, space="DRAM")
                )
                vins = [dram.tile([4 * P, D], BF, name=f"vin{h}") for h in range(2)]
                vgs = [
                    dram.tile([8 * P, D], BF, name=f"vg{h}") for h in range(2)
                ]
                aps = actx.enter_context(
                    tc.tile_pool(name="aps", bufs=8, space="PSUM")
                )
                vsb = actx.enter_context(tc.tile_pool(name="vsb", bufs=NQB))
                for half in range(2):
                    pss = [
                        [aps.tile([P, 512], F32, tag="ps", name="ps") for _ in range(2)]
                        for _ in range(4)
                    ]
                    for d in range(ND):
                        for si in range(4):
                            s = half * 4 + si
                            for nt in range(2):
                                nc.tensor.matmul(
                                    pss[si][nt][:],
                                    XQ[d][:, s * P : (s + 1) * P],
                                    WV[d][:, nt * 512 : (nt + 1) * 512],
                                    start=(d == 0),
                                    stop=(d == ND - 1),
                                )
                    for si in range(4):
                        vt = vsb.tile([P, D], BF, name="vsb")
                        for nt in range(2):
                            nc.scalar.activation(
                                vt[:, nt * 512 : (nt + 1) * 512],
                                pss[si][nt][:],
                                AF.Copy,
                            )
                        nc.gpsimd.dma_start(vins[half][si * P : (si + 1) * P, :], vt[:])
                    nc.gpsimd.collective_compute(
                        "AllGather",
                        mybir.AluOpType.bypass,
                        replica_groups=[[0, 1], [2, 3], [4, 5], [6, 7]],
                        ins=[vins[half].opt()],
                        outs=[vgs[half].opt()],
                    )
                # readback in ascending global-block order (attend order);
                # gather h rows = my 4 blocks then peer's 4 blocks
                for g in range(NSK):
                    h, i = g // 8, VG_POS[g] % 8
                    vt = v_pool.tile([P, D], BF, name="v")
                    (nc.sync if g % 2 == 0 else nc.scalar).dma_start(
                        vt[:], vgs[h][i * P : (i + 1) * P, :]
                    )
                    V[g] = vt

                # ---- QT, e-outer (xq fully resident by now) ----
                for e in range(NE):
                    qts = qt_pool.tile([P, SQH], BF, name="qt")
                    for h in range(2):
                        qp = aps.tile([P, 512], F32, tag="ps", name="ps")
                        for d in range(ND):
                            nc.tensor.matmul(
                                qp[:],
                                WQ[d][:, e * P : (e + 1) * P],
                                XQ[d][:, h * 512 : (h + 1) * 512],
                                start=(d == 0),
                                stop=(d == ND - 1),
                            )
                        nc.scalar.activation(
                            qts[:, h * 512 : (h + 1) * 512], qp[:], AF.Copy
                        )
                    QT.append(qts)
            else:
                # ---- QT first (d-outer, 2 q-passes, streams xq+wq) ----
                aps = actx.enter_context(
                    tc.tile_pool(name="aps", bufs=8, space="PSUM")
                )
                for h in range(2):
                    pss = [aps.tile([P, 512], F32, tag="ps", name="ps") for _ in range(NE)]
                    for d in range(ND):
                        for e in range(NE):
                            nc.tensor.matmul(
                                pss[e][:],
                                WQ[d][:, e * P : (e + 1) * P],
                                XQ[d][:, h * 512 : (h + 1) * 512],
                                start=(d == 0),
                                stop=(d == ND - 1),
                            )
                    for e in range(NE):
                        if h == 0:
                            QT.append(qt_pool.tile([P, SQH], BF, name="qt"))
                        nc.scalar.activation(
                            QT[e][:, h * 512 : (h + 1) * 512], pss[e][:], AF.Copy
                        )
                # ---- V' for all 16 tiles from xT, s-outer ----
                for s in range(NSK):
                    vt = v_pool.tile([P, D], BF, name="v")
                    pss = [aps.tile([P, 512], F32, tag="ps", name="ps") for _ in range(2)]
                    for d in range(ND):
                        for nt in range(2):
                            nc.tensor.matmul(
                                pss[nt][:],
                                XT[d][:, s * P : (s + 1) * P],
                                WV[d][:, nt * 512 : (nt + 1) * 512],
                                start=(d == 0),
                                stop=(d == ND - 1),
                            )
                    for nt in range(2):
                        nc.scalar.activation(
                            vt[:, nt * 512 : (nt + 1) * 512], pss[nt][:], AF.Copy
                        )
                    V[s] = vt

        # ---------------- phase B: scores + attend ----------------
        with ExitStack() as bctx:
            maskp = bctx.enter_context(tc.tile_pool(name="mask", bufs=2))
            smp = bctx.enter_context(tc.tile_pool(name="sm", bufs=2))
            expA = bctx.enter_context(tc.tile_pool(name="expA", bufs=8))
            expB = bctx.enter_context(tc.tile_pool(name="expB", bufs=8))
            statp = bctx.enter_context(tc.tile_pool(name="stat", bufs=2))
            outp = bctx.enter_context(tc.tile_pool(name="out", bufs=2))
            ps_s = bctx.enter_context(tc.tile_pool(name="ps_s", bufs=4, space="PSUM"))
            ps_a = bctx.enter_context(tc.tile_pool(name="ps_a", bufs=3, space="PSUM"))
            denp = bctx.enter_context(tc.tile_pool(name="den", bufs=1, space="PSUM"))
            EXP = [None] * NSK

            def emit_scores(g):
                t0 = g // 2
                span = (NQB - t0) * P
                qoff = t0 * P
                # additive causal mask for the slot-t0 q-slice of this tile
                mk = maskp.tile([P, P], F32, tag="mk", name="mk")
                nc.vector.tensor_scalar(
                    mk[:],
                    qpos[:, qoff : qoff + P],
                    kposc[:, g : g + 1],
                    NEG_BIG,
                    op0=Alu.is_lt,
                    op1=Alu.mult,
                )
                pool = expA if span > 512 else expB
                ex = pool.tile([P, max(span, 512)], BF, name="ex")
                EXP[g] = ex
                for off in range(0, span, 512):
                    w = min(512, span - off)
                    ps = ps_s.tile([P, 512], F32, tag="ps", name="ps")
                    for e in range(NE):
                        nc.tensor.matmul(
                            ps[:, 0:w],
                            XT[e][:, g * P : (g + 1) * P],
                            QT[e][:, qoff + off : qoff + off + w],
                            start=(e == 0),
                            stop=(e == NE - 1),
                        )
                    if off == 0:
                        sm = smp.tile([P, P], F32, tag="sm", name="sm")
                        nc.vector.tensor_tensor(sm[:], ps[:, 0:P], mk[:], op=Alu.add)
                        nc.scalar.activation(
                            ex[:, 0:P], sm[:], AF.Exp, scale=float(SCALE)
                        )
                        if w > P:
                            nc.scalar.activation(
                                ex[:, P:w], ps[:, P:w], AF.Exp, scale=float(SCALE)
                            )
                    else:
                        nc.scalar.activation(
                            ex[:, off : off + w],
                            ps[:, 0:w],
                            AF.Exp,
                            scale=float(SCALE),
                        )

            def emit_attend(t):
                nk = 2 * t + 2
                dps = denp.tile([P, 1], F32, tag="dp", name="dp")
                pas = [ps_a.tile([P, 512], F32, tag="pa", name="pa") for _ in range(2)]
                for g2 in range(nk):
                    lt = EXP[g2][:, (t - g2 // 2) * P : (t - g2 // 2 + 1) * P]
                    nc.tensor.matmul(
                        dps[:], lt, ones1, start=(g2 == 0), stop=(g2 == nk - 1)
                    )
                    for nt in range(2):
                        nc.tensor.matmul(
                            pas[nt][:],
                            lt,
                            V[g2][:, nt * 512 : (nt + 1) * 512],
                            start=(g2 == 0),
                            stop=(g2 == nk - 1),
                        )
                rinv = statp.tile([P, 1], F32, tag="ri", name="ri")
                nc.vector.reciprocal(rinv[:], dps[:])
                ob = outp.tile([P, D], F32, tag="ob", name="ob")
                for nt in range(2):
                    nc.scalar.activation(
                        ob[:, nt * 512 : (nt + 1) * 512],
                        pas[nt][:],
                        AF.Copy,
                        scale=rinv[:],
                    )
                    nc.sync.dma_start(
                        out_d[t][:, nt * 512 : (nt + 1) * 512],
                        ob[:, nt * 512 : (nt + 1) * 512],
                    )

            # lagged interleave: A(t) emitted after S(2t+6), so the PE
            # never reaches an attend before its gathered V has landed
            for g in range(NSK):
                emit_scores(g)
                if g >= 6 and g % 2 == 0:
                    emit_attend((g - 6) // 2)
            for t in range(NQB - 3, NQB):
                emit_attend(t)


def build_nc():
    """Build + compile the SPMD Bass program (cached)."""
    global _NC
    if _NC is not None:
        return _NC
    from concourse import bacc, mybir
    import concourse.tile as tile

    BF = mybir.dt.bfloat16
    F32 = mybir.dt.float32

    nc = bacc.Bacc(
        "TRN2",
        target_bir_lowering=False,
        debug=False,
        enable_asserts=False,
        num_devices=8,
    )
    dr = {}

    def din(name, shape, dt):
        dr[name] = nc.dram_tensor(name, shape, dt, kind="ExternalInput").ap()

    din("xT", (ND, P, S), BF)
    din("xq", (ND, P, SQH), BF)
    din("wq2", (ND, P, D), BF)
    din("wvT", (ND, P, D), BF)
    din("cpak", (P, CPW), F32)
    din("cbf", (P, 8), BF)
    out_d = nc.dram_tensor("out_c", (NQB, P, D), F32, kind="ExternalOutput").ap()

    with tile.TileContext(nc) as tc:
        _emit(nc, tc, dr, out_d)
    nc.compile()
    _NC = nc
    return nc


def make_in_maps(x, Wq, bq, Wk, bk, Wv, bv, Wo, bo):
    """Host-side sharding: per-core input dicts (bf16 compute operands)."""
    bf16 = ml_dtypes.bfloat16
    f32 = np.float32

    # host-fused weights (f32 GEMMs, exact up to fp32):
    #   scores = (x Wq^T)(x Wk^T)^T = x (Wq^T Wk) x^T       -> Wqk
    #   out    = softmax(..) (x Wv^T) Wo^T = softmax(..) x (Wo Wv)^T
    # Requires bq = bk = 0 (guaranteed by the problem spec).
    Wqk = Wq.T.astype(np.float32) @ Wk.astype(np.float32)  # [d1, d2]
    Wvo = Wo.astype(np.float32) @ Wv.astype(np.float32)  # [eo, d]
    wq2 = np.ascontiguousarray(Wqk.reshape(ND, P, D)).astype(bf16)
    wvT = np.ascontiguousarray(Wvo.T.reshape(ND, P, D)).astype(bf16)
    kposc = (np.arange(NSK, dtype=f32) * P)[None, :] + np.arange(P, dtype=f32)[
        :, None
    ]  # [P, NSK]: kposc[p, g] = g*128 + p
    cbf = np.ones((P, 8), dtype=bf16)

    in_maps = []
    for c in range(8):
        b, j = c // 2, c % 2
        blocks = J_BLOCKS[j]
        xTb = np.ascontiguousarray(x[b].T)  # [D, S] natural key order
        qcols = np.concatenate(
            [np.r_[P * g : P * (g + 1)] for g in blocks]
        )
        xqb = np.ascontiguousarray(xTb[:, qcols])  # [D, SQH]
        qpos = np.broadcast_to(qcols.astype(f32), (P, SQH))
        cpak = np.concatenate([qpos, kposc], axis=1)
        in_maps.append(
            {
                "xT": xTb.reshape(ND, P, S).astype(bf16),
                "xq": xqb.reshape(ND, P, SQH).astype(bf16),
                "wq2": wq2,
                "wvT": wvT,
                "cpak": np.ascontiguousarray(cpak.astype(f32)),
                "cbf": cbf,
            }
        )
    return in_maps


def assemble_out(results, bvo):
    out = np.empty((B, S, D), dtype=np.float32)
    for c in range(8):
        b, j = c // 2, c % 2
        blocks = J_BLOCKS[j]
        oc = results[c]["out_c"]  # (8, 128, 1024)
        for t, g in enumerate(blocks):
            out[b, P * g : P * (g + 1), :] = oc[t]
    if bvo is not None:
        out += bvo[None, None, :]
    return out


def kernel(x, Wq, bq, Wk, bk, Wv, bv, Wo, bo):
    from concourse.bass_utils import run_bass_kernel_spmd

    nc = build_nc()
    in_maps = make_in_maps(x, Wq, bq, Wk, bk, Wv, bv, Wo, bo)
    res = run_bass_kernel_spmd(nc, in_maps, core_ids=list(range(8)))
    bvo = Wo.astype(np.float32) @ bv.astype(np.float32) + bo.astype(np.float32)
    return assemble_out(res.results, bvo)


# revision 9
# speedup vs baseline: 1.3070x; 1.1325x over previous
"""Causal self-attention (single head) on 8 TRN2 NeuronCores.

Sharding: data-parallel over batch (4) x query-interleave (2).
Core c handles batch b = c//2 and the 8 query blocks (128 q each)
J_BLOCKS[c%2]; slot t's block g is in {2t, 2t+1}, so covering key
tiles 0..2t+1 (natural global order) is uniform across cores and the
causal mask beyond the static structure is data-driven.

Algorithm per core (all matmuls bf16 with f32 PSUM):
  QT[d2, q]  = Wqk^T-projected queries (Wqk = Wq^T Wk host-fused), so
               scores = QT^T x_k^T needs no on-chip K projection.
  V'[k, eo]  = x Wvo^T (Wvo = Wo Wv host-fused): attended @ V' IS the
               final output projection.
  scoresT[k, q] computed per key tile kt with q spanning all slots
               that cover kt -> wide matmuls, and softmax weights come
               out already in [k, q] layout: no transposes at all.
  softmax    = exp without max subtraction (scores ~ N(0,1), safe in
               f32), denominator via ones-column matmul, normalization
               folded into the attended PSUM eviction scale.

USE_CC: V' is computed only for this core's own 8 key blocks and
pair-wise AllGathered (cores 2b/2b+1), halving the biggest matmul.
"""

from contextlib import ExitStack

import numpy as np
import ml_dtypes

USE_CC = True

B, S, D = 4, 2048, 1024
P = 128
ND = D // P  # 8 contraction chunks
NE = D // P  # 8 feature chunks
NSK = S // P  # 16 key tiles
NQB = 8  # query blocks per core
SQH = NQB * P  # 1024 queries per core
J_BLOCKS = (
    [0, 3, 4, 7, 8, 11, 12, 15],
    [1, 2, 5, 6, 9, 10, 13, 14],
)
# per-half allgather row order: [my 4 blocks (rank 0) | peer 4 (rank 1)]
VG_BLOCK = (
    J_BLOCKS[0][:4] + J_BLOCKS[1][:4] + J_BLOCKS[0][4:] + J_BLOCKS[1][4:]
)
VG_POS = {g: i for i, g in enumerate(VG_BLOCK)}
SCALE = 1.0 / np.sqrt(np.float32(D))  # 1/32
NEG_BIG = -1.0e30
CPW = SQH + NSK  # packed f32 consts width

_NC = None


def _emit(nc, tc, dr, out_d):
    from concourse import mybir

    BF = mybir.dt.bfloat16
    F32 = mybir.dt.float32
    AF = mybir.ActivationFunctionType
    Alu = mybir.AluOpType

    with ExitStack() as ctx:
        const = ctx.enter_context(tc.tile_pool(name="const", bufs=1))
        cpak = const.tile([P, CPW], F32)
        nc.gpsimd.dma_start(cpak[:], dr["cpak"])
        qpos = cpak[:, 0:SQH]
        kposc = cpak[:, SQH : SQH + NSK]
        cbf = const.tile([P, 8], BF)
        nc.gpsimd.dma_start(cbf[:], dr["cbf"])
        ones1 = cbf[:, 0:1]

        # persistent activation storage
        xq_pool = ctx.enter_context(tc.tile_pool(name="xq", bufs=ND))
        xt_pool = ctx.enter_context(tc.tile_pool(name="xt", bufs=ND))
        qt_pool = ctx.enter_context(tc.tile_pool(name="qt", bufs=NE))
        v_pool = ctx.enter_context(tc.tile_pool(name="v", bufs=NSK))
        XQ, XT, QT, V = [None] * ND, [None] * ND, [], [None] * NSK

        # ---------------- phase A ----------------
        with ExitStack() as actx:
            wv_pool = actx.enter_context(tc.tile_pool(name="wv", bufs=ND))
            wq_pool = actx.enter_context(tc.tile_pool(name="wq", bufs=ND))
            WV, WQ = [None] * ND, [None] * ND

            # input streams on the 3 DMA rings (sync/scalar/gpsimd),
            # interleaved by need order (V d-step d needs xq[d]+wv[d]).
            # CRITICAL: the scalar (Activation) engine gets only a short
            # non-blocking prefix of triggers -- anything that can block
            # on a ring semaphore would delay the V/QT PSUM evictions.
            RINGS = (nc.sync, nc.scalar, nc.gpsimd)

            def load(lst, d, pool, w, key, ring):
                t = pool.tile([P, w], BF, name=key)
                RINGS[ring].dma_start(t[:], dr[key][d])
                lst[d] = t

            load(XQ, 0, xq_pool, SQH, "xq", 0)
            load(WV, 0, wv_pool, D, "wvT", 1)
            load(WV, 2, wv_pool, D, "wvT", 0)
            load(XQ, 1, xq_pool, SQH, "xq", 1)
            load(WV, 1, wv_pool, D, "wvT", 2)
            load(XQ, 3, xq_pool, SQH, "xq", 0)
            load(XQ, 4, xq_pool, SQH, "xq", 1)
            load(WV, 4, wv_pool, D, "wvT", 2)
            load(WV, 5, wv_pool, D, "wvT", 0)
            load(WV, 3, wv_pool, D, "wvT", 1)
            load(WV, 7, wv_pool, D, "wvT", 2)
            load(XQ, 6, xq_pool, SQH, "xq", 0)
            load(XQ, 7, xq_pool, SQH, "xq", 1)
            load(XQ, 2, xq_pool, SQH, "xq", 2)
            load(WV, 6, wv_pool, D, "wvT", 1)
            load(XQ, 5, xq_pool, SQH, "xq", 2)
            for d in range(ND):
                load(WQ, d, wq_pool, D, "wq2", d % 2)
            for d in range(ND):
                load(XT, d, xt_pool, S, "xT", 0)

            if USE_CC:
                # ---- V' for my 8 blocks (from xq), d-outer in 2 passes.
                # Each pass's 4 blocks, paired with the peer's, are exactly
                # global key tiles 0..7 (pass 0) / 8..15 (pass 1), so each
                # pass feeds its own AllGather: the first gather is in
                # flight while pass 1 + QT still compute. ----
                dram = actx.enter_context(
                    tc.tile_pool(name="dram", bufs=1, space="DRAM")
                )
                vins = [dram.tile([4 * P, D], BF, name=f"vin{h}") for h in range(2)]
                vgs = [
                    dram.tile([8 * P, D], BF, name=f"vg{h}") for h in range(2)
                ]
                aps = actx.enter_context(
                    tc.tile_pool(name="aps", bufs=8, space="PSUM")
                )
                vsb = actx.enter_context(tc.tile_pool(name="vsb", bufs=NQB))
                for half in range(2):
                    pss = [
                        [aps.tile([P, 512], F32, tag="ps", name="ps") for _ in range(2)]
                        for _ in range(4)
                    ]
                    for d in range(ND):
                        for si in range(4):
                            s = half * 4 + si
                            for nt in range(2):
                                nc.tensor.matmul(
                                    pss[si][nt][:],
                                    XQ[d][:, s * P : (s + 1) * P],
                                    WV[d][:, nt * 512 : (nt + 1) * 512],
                                    start=(d == 0),
                                    stop=(d == ND - 1),
                                )
                    for si in range(4):
                        vt = vsb.tile([P, D], BF, name="vsb")
                        for nt in range(2):
                            nc.scalar.activation(
                                vt[:, nt * 512 : (nt + 1) * 512],
                                pss[si][nt][:],
                                AF.Copy,
                            )
                        nc.gpsimd.dma_start(vins[half][si * P : (si + 1) * P, :], vt[:])
                    nc.gpsimd.collective_compute(
                        "AllGather",
                        mybir.AluOpType.bypass,
                        replica_groups=[[0, 1], [2, 3], [4, 5], [6, 7]],
                        ins=[vins[half].opt()],
                        outs=[vgs[half].opt()],
                    )
                # readback in ascending global-block order (attend order);
                # gather h rows = my 4 blocks then peer's 4 blocks
                for g in range(NSK):
                    h, i = g // 8, VG_POS[g] % 8
                    vt = v_pool.tile([P, D], BF, name="v")
                    (nc.sync if g % 2 == 0 else nc.gpsimd).dma_start(
                        vt[:], vgs[h][i * P : (i + 1) * P, :]
                    )
                    V[g] = vt

                # ---- QT, e-outer (xq fully resident by now) ----
                for e in range(NE):
                    qts = qt_pool.tile([P, SQH], BF, name="qt")
                    for h in range(2):
                        qp = aps.tile([P, 512], F32, tag="ps", name="ps")
                        for d in range(ND):
                            nc.tensor.matmul(
                                qp[:],
                                WQ[d][:, e * P : (e + 1) * P],
                                XQ[d][:, h * 512 : (h + 1) * 512],
                                start=(d == 0),
                                stop=(d == ND - 1),
                            )
                        nc.scalar.activation(
                            qts[:, h * 512 : (h + 1) * 512], qp[:], AF.Copy
                        )
                    QT.append(qts)
            else:
                # ---- QT first (d-outer, 2 q-passes, streams xq+wq) ----
                aps = actx.enter_context(
                    tc.tile_pool(name="aps", bufs=8, space="PSUM")
                )
                for h in range(2):
                    pss = [aps.tile([P, 512], F32, tag="ps", name="ps") for _ in range(NE)]
                    for d in range(ND):
                        for e in range(NE):
                            nc.tensor.matmul(
                                pss[e][:],
                                WQ[d][:, e * P : (e + 1) * P],
                                XQ[d][:, h * 512 : (h + 1) * 512],
                                start=(d == 0),
                                stop=(d == ND - 1),
                            )
                    for e in range(NE):
                        if h == 0:
                            QT.append(qt_pool.tile([P, SQH], BF, name="qt"))
                        nc.scalar.activation(
                            QT[e][:, h * 512 : (h + 1) * 512], pss[e][:], AF.Copy
                        )
                # ---- V' for all 16 tiles from xT, s-outer ----
                for s in range(NSK):
                    vt = v_pool.tile([P, D], BF, name="v")
                    pss = [aps.tile([P, 512], F32, tag="ps", name="ps") for _ in range(2)]
                    for d in range(ND):
                        for nt in range(2):
                            nc.tensor.matmul(
                                pss[nt][:],
                                XT[d][:, s * P : (s + 1) * P],
                                WV[d][:, nt * 512 : (nt + 1) * 512],
                                start=(d == 0),
                                stop=(d == ND - 1),
                            )
                    for nt in range(2):
                        nc.scalar.activation(
                            vt[:, nt * 512 : (nt + 1) * 512], pss[nt][:], AF.Copy
                        )
                    V[s] = vt

        # ---------------- phase B: scores + attend ----------------
        with ExitStack() as bctx:
            maskp = bctx.enter_context(tc.tile_pool(name="mask", bufs=2))
            smp = bctx.enter_context(tc.tile_pool(name="sm", bufs=2))
            expA = bctx.enter_context(tc.tile_pool(name="expA", bufs=8))
            expB = bctx.enter_context(tc.tile_pool(name="expB", bufs=8))
            statp = bctx.enter_context(tc.tile_pool(name="stat", bufs=2))
            outp = bctx.enter_context(tc.tile_pool(name="out", bufs=2))
            ps_s = bctx.enter_context(tc.tile_pool(name="ps_s", bufs=4, space="PSUM"))
            ps_a = bctx.enter_context(tc.tile_pool(name="ps_a", bufs=3, space="PSUM"))
            denp = bctx.enter_context(tc.tile_pool(name="den", bufs=1, space="PSUM"))
            EXP = [None] * NSK

            def emit_scores(g):
                t0 = g // 2
                span = (NQB - t0) * P
                qoff = t0 * P
                # additive causal mask for the slot-t0 q-slice of this tile
                mk = maskp.tile([P, P], F32, tag="mk", name="mk")
                nc.vector.tensor_scalar(
                    mk[:],
                    qpos[:, qoff : qoff + P],
                    kposc[:, g : g + 1],
                    NEG_BIG,
                    op0=Alu.is_lt,
                    op1=Alu.mult,
                )
                pool = expA if span > 512 else expB
                ex = pool.tile([P, max(span, 512)], BF, name="ex")
                EXP[g] = ex
                for off in range(0, span, 512):
                    w = min(512, span - off)
                    ps = ps_s.tile([P, 512], F32, tag="ps", name="ps")
                    for e in range(NE):
                        nc.tensor.matmul(
                            ps[:, 0:w],
                            XT[e][:, g * P : (g + 1) * P],
                            QT[e][:, qoff + off : qoff + off + w],
                            start=(e == 0),
                            stop=(e == NE - 1),
                        )
                    if off == 0:
                        sm = smp.tile([P, P], F32, tag="sm", name="sm")
                        nc.vector.tensor_tensor(sm[:], ps[:, 0:P], mk[:], op=Alu.add)
                        nc.scalar.activation(
                            ex[:, 0:P], sm[:], AF.Exp, scale=float(SCALE)
                        )
                        if w > P:
                            nc.scalar.activation(
                                ex[:, P:w], ps[:, P:w], AF.Exp, scale=float(SCALE)
                            )
                    else:
                        nc.scalar.activation(
                            ex[:, off : off + w],
                            ps[:, 0:w],
                            AF.Exp,
                            scale=float(SCALE),
                        )

            def emit_attend(t):
                nk = 2 * t + 2
                dps = denp.tile([P, 1], F32, tag="dp", name="dp")
                pas = [ps_a.tile([P, 512], F32, tag="pa", name="pa") for _ in range(2)]
                for g2 in range(nk):
                    lt = EXP[g2][:, (t - g2 // 2) * P : (t - g2 // 2 + 1) * P]
                    nc.tensor.matmul(
                        dps[:], lt, ones1, start=(g2 == 0), stop=(g2 == nk - 1)
                    )
                    for nt in range(2):
                        nc.tensor.matmul(
                            pas[nt][:],
                            lt,
                            V[g2][:, nt * 512 : (nt + 1) * 512],
                            start=(g2 == 0),
                            stop=(g2 == nk - 1),
                        )
                rinv = statp.tile([P, 1], F32, tag="ri", name="ri")
                nc.vector.reciprocal(rinv[:], dps[:])
                ob = outp.tile([P, D], F32, tag="ob", name="ob")
                for nt in range(2):
                    nc.scalar.activation(
                        ob[:, nt * 512 : (nt + 1) * 512],
                        pas[nt][:],
                        AF.Copy,
                        scale=rinv[:],
                    )
                    nc.sync.dma_start(
                        out_d[t][:, nt * 512 : (nt + 1) * 512],
                        ob[:, nt * 512 : (nt + 1) * 512],
                    )

            # lagged interleave: A(t) emitted after S(2t+6), so the PE
            # never reaches an attend before its gathered V has landed
            for g in range(NSK):
                emit_scores(g)
                if g >= 6 and g % 2 == 0:
                    emit_attend((g - 6) // 2)
            for t in range(NQB - 3, NQB):
                emit_attend(t)


def build_nc():
    """Build + compile the SPMD Bass program (cached)."""
    global _NC
    if _NC is not None:
        return _NC
    from concourse import bacc, mybir
    import concourse.tile as tile

    BF = mybir.dt.bfloat16
    F32 = mybir.dt.float32

    nc = bacc.Bacc(
        "TRN2",
        target_bir_lowering=False,
        debug=False,
        enable_asserts=False,
        num_devices=8,
    )
    dr = {}

    def din(name, shape, dt):
        dr[name] = nc.dram_tensor(name, shape, dt, kind="ExternalInput").ap()

    din("xT", (ND, P, S), BF)
    din("xq", (ND, P, SQH), BF)
    din("wq2", (ND, P, D), BF)
    din("wvT", (ND, P, D), BF)
    din("cpak", (P, CPW), F32)
    din("cbf", (P, 8), BF)
    out_d = nc.dram_tensor("out_c", (NQB, P, D), F32, kind="ExternalOutput").ap()

    with tile.TileContext(nc) as tc:
        _emit(nc, tc, dr, out_d)
    nc.compile()
    _NC = nc
    return nc


def make_in_maps(x, Wq, bq, Wk, bk, Wv, bv, Wo, bo):
    """Host-side sharding: per-core input dicts (bf16 compute operands)."""
    bf16 = ml_dtypes.bfloat16
    f32 = np.float32

    # host-fused weights (f32 GEMMs, exact up to fp32):
    #   scores = (x Wq^T)(x Wk^T)^T = x (Wq^T Wk) x^T       -> Wqk
    #   out    = softmax(..) (x Wv^T) Wo^T = softmax(..) x (Wo Wv)^T
    # Requires bq = bk = 0 (guaranteed by the problem spec).
    Wqk = Wq.T.astype(np.float32) @ Wk.astype(np.float32)  # [d1, d2]
    Wvo = Wo.astype(np.float32) @ Wv.astype(np.float32)  # [eo, d]
    wq2 = np.ascontiguousarray(Wqk.reshape(ND, P, D)).astype(bf16)
    wvT = np.ascontiguousarray(Wvo.T.reshape(ND, P, D)).astype(bf16)
    kposc = (np.arange(NSK, dtype=f32) * P)[None, :] + np.arange(P, dtype=f32)[
        :, None
    ]  # [P, NSK]: kposc[p, g] = g*128 + p
    cbf = np.ones((P, 8), dtype=bf16)

    in_maps = []
    for c in range(8):
        b, j = c // 2, c % 2
        blocks = J_BLOCKS[j]
        xTb = np.ascontiguousarray(x[b].T)  # [D, S] natural key order
        qcols = np.concatenate(
            [np.r_[P * g : P * (g + 1)] for g in blocks]
        )
        xqb = np.ascontiguousarray(xTb[:, qcols])  # [D, SQH]
        qpos = np.broadcast_to(qcols.astype(f32), (P, SQH))
        cpak = np.concatenate([qpos, kposc], axis=1)
        in_maps.append(
            {
                "xT": xTb.reshape(ND, P, S).astype(bf16),
                "xq": xqb.reshape(ND, P, SQH).astype(bf16),
                "wq2": wq2,
                "wvT": wvT,
                "cpak": np.ascontiguousarray(cpak.astype(f32)),
                "cbf": cbf,
            }
        )
    return in_maps


def assemble_out(results, bvo):
    out = np.empty((B, S, D), dtype=np.float32)
    for c in range(8):
        b, j = c // 2, c % 2
        blocks = J_BLOCKS[j]
        oc = results[c]["out_c"]  # (8, 128, 1024)
        for t, g in enumerate(blocks):
            out[b, P * g : P * (g + 1), :] = oc[t]
    if bvo is not None:
        out += bvo[None, None, :]
    return out


def kernel(x, Wq, bq, Wk, bk, Wv, bv, Wo, bo):
    from concourse.bass_utils import run_bass_kernel_spmd

    nc = build_nc()
    in_maps = make_in_maps(x, Wq, bq, Wk, bk, Wv, bv, Wo, bo)
    res = run_bass_kernel_spmd(nc, in_maps, core_ids=list(range(8)))
    bvo = Wo.astype(np.float32) @ bv.astype(np.float32) + bo.astype(np.float32)
    return assemble_out(res.results, bvo)
